# revision 2
# baseline (speedup 1.0000x reference)
"""GridRNN Trainium2 kernel.

Problem: 2-D grid RNN, B=4, S=T=128, H=256, D=3 depths.
  hx[d][b,i,j] = tanh(xin @ Wx_ih[d].T + bx_ih[d] + hx[d][b,i-1,(j-1)%T] @ Wx_hh[d].T + bx_hh[d])
  hy[d][b,i,j] = tanh(yin @ Wy_ih[d].T + by_ih[d] + hy[d][b,i,j-1]     @ Wy_hh[d].T + by_hh[d])
  (xin/yin = src/trg broadcast at d=0, previous depth's hx/hy for d>0)
  out = stack([hx[D-1], hy[D-1]], axis=-2)   # [B,S,T,2,H]

Key structure: the x-chain and y-chain never mix across depths -> 8 cores =
4 batches x 2 chains.  The x-chain's diagonal dependence hx[i-1,(j-1)%T] is
removed by shearing: u_i[c] = hx[i,(i+c)%T] turns it into a plain carry
u_{i-1}[c], identical in form to the y-chain.  One SPMD program runs on all
8 cores; only the input data (seed, weights) differs per core.  The host
unshears the x outputs and transposes the y outputs.

On-chip layout per step: state u kept as [128(part)=H%128, 2(k=H/128), V=128]
(H on partitions as two k-tiles), in BF16: the PE runs bf16 matmuls at 1
cycle/row vs fp32's 4 (PSUM still accumulates fp32, biases stay fp32 via the
ACT bias port, tolerance is 2e-2).  Recurrence out[Hout,V] = W.T tiles (lhsT)
@ state tiles (rhs), accumulated in PSUM, tanh via ScalarE with per-partition
bias writing bf16 states.  Depth-0's input term depends only on the step
index, so it is precomputed once as fp32 columns and folded into the tanh
bias.  Output leaves in bf16; the host upcasts.

This walrus build allows only ONE sync-wait per hardware instruction, so the
kernel is structured to keep Tile's emitted waits at <=1 everywhere: all
constants arrive in two packed DMAs (bf16 weights blob + small fp32 blob),
absorber ops fold DMA-queue semaphores into each engine's vector clock,
outputs accumulate in one big SBUF tile and leave in a few large DMAs (few
DMA lanes -> short tail drain).
"""

import numpy as np
import ml_dtypes

import concourse.bass as bass
import concourse.tile as tile
from concourse import mybir
from concourse.bass_utils import run_bass_kernel_spmd

B, S, T, H, D = 4, 128, 128, 256, 3
P = 128          # partitions
K = H // P       # 2 k-tiles of H on partitions
F32 = mybir.dt.float32
BF16 = mybir.dt.bfloat16
NPBF16 = np.dtype(ml_dtypes.bfloat16)
TANH = mybir.ActivationFunctionType.Tanh

# wblob (bf16) column layout
W0 = 0                    # wihT: (d, k, m) -> W0 + (d*K+k)*H + m*P
W1 = W0 + D * K * H       # whhT
S0 = W1 + D * K * H       # seed row (this partition's step row)
I0 = S0 + H               # identity
WCW = I0 + P

# cblob (fp32) column layout: bias cols (d, m) -> d*K + m
CCW = D * K

OCHUNK = 64

_cache = {}


def _patched_drain_and_barrier(self, tick_clock, wait_clock):
    """Replacement for TileContext._drain_and_barrier.

    This walrus build lowers at most ONE sync-wait per instruction; the stock
    tail drain carries one wait per active proc.  Semantically the waits only
    need to complete before the final barrier's semaphore cleanup, so spread
    them over single-wait NOPs on the sync engine after the drain.
    """
    drain_inst = self.nc.sync.drain()
    wait_clock.add_sem_waits(
        drain_inst.ins, tile.ScopedClock({None: tick_clock.global_clock})
    )
    ins = drain_inst.ins
    si = ins.sync_info
    if si is not None and len(si.on_wait) > 1:
        waits = list(si.on_wait)
        ins.sync_info = mybir.SyncInfo(on_wait=[waits[0]],
                                       on_update=list(si.on_update))
        for w in waits[1:]:
            nop = self.nc.sync.nop(nofuse=True)
            nop.ins.sync_info = mybir.SyncInfo(on_wait=[w], on_update=[])

    self.nc.all_engine_barrier()
    assert self.sems is not None
    popped = self.nc._tile_sem_poison_stack.pop()
    assert popped is self._sem_poison
    self.nc.clear_and_free_semaphores(list(self.sems.allocated().values()))
    self.nc.all_engine_barrier()


tile.TileContext._drain_and_barrier = _patched_drain_and_barrier


def _build():
    nc = bass.Bass(trn_type="TRN2")

    wblob = nc.dram_tensor("wblob", [P, WCW], BF16, kind="ExternalInput")
    cblob = nc.dram_tensor("cblob", [P, CCW], F32, kind="ExternalInput")
    # DRAM layout mirrors SBUF exactly ([p, s, k, v]) so the output DMA is
    # 128 fully-contiguous runs; the host reassembles H = k*128+p.
    out = nc.dram_tensor("out", [P, S, K, T], BF16, kind="ExternalOutput")
    out_c = out[:, :, :, :]

    with tile.TileContext(nc) as tc:
        with (
            tc.tile_pool(name="consts", bufs=1) as consts,
            tc.tile_pool(name="u0p", bufs=4) as u0p,
            tc.tile_pool(name="u1p", bufs=4) as u1p,
            tc.tile_pool(name="ps0", bufs=2, space="PSUM") as ps0p,
            tc.tile_pool(name="ps1", bufs=2, space="PSUM") as ps1p,
            tc.tile_pool(name="ps2", bufs=2, space="PSUM") as ps2p,
            tc.tile_pool(name="psi", bufs=1, space="PSUM") as psip,
        ):
            wb = consts.tile([P, WCW], BF16)
            cb = consts.tile([P, CCW], F32)
            nc.gpsimd.dma_start(out=wb, in_=wblob[:, :])
            nc.gpsimd.dma_start(out=cb, in_=cblob[:, :])

            def wih(d, k, m):
                c = W0 + (d * K + k) * H + m * P
                return wb[:, c:c + P]

            def whh(d, k, m):
                c = W1 + (d * K + k) * H + m * P
                return wb[:, c:c + P]

            def bias(d, m):
                c = d * K + m
                return cb[:, c:c + 1]

            seed_sb = wb[:, S0:S0 + H]
            ident = wb[:, I0:I0 + P]

            zeros = consts.tile([P, K, T], BF16)
            nc.vector.memset(zeros, 0.0)
            # ScalarE absorber: folds the const-DMA semaphore into ACT's clock
            scr = consts.tile([P, 4], F32)
            nc.scalar.copy(out=scr[:, 0:1], in_=bias(0, 0))
            # PE absorber + warmup: folds the wblob-DMA semaphore into PE's clock
            dummy = psip.tile([32, 32], F32, tag="init")
            nc.tensor.matmul(dummy[:, :], lhsT=wb[0:32, 0:32], rhs=wb[0:32, 0:32],
                             start=True, stop=True)

            # ---- seedT[k] = seed[:, k*128:(k+1)*128].T  (PE transpose, bf16)
            seedT_sb = consts.tile([P, K, S], BF16)
            pst = psip.tile([P, K, S], BF16, tag="init")
            for k in range(K):
                nc.tensor.transpose(pst[:, k, :], seed_sb[:, k * P:(k + 1) * P], ident)
            nc.vector.tensor_copy(seedT_sb, pst)

            # ---- pre0[:, m, s] = (W_ih[0] @ seed[s] + bsum[0])[m*128+p]  (fp32)
            pre0_sb = consts.tile([P, K, S], F32)
            psp = psip.tile([P, K, S], F32, tag="init2")
            for m in range(K):
                for k in range(K):
                    nc.tensor.matmul(
                        psp[:, m, :], lhsT=wih(0, k, m), rhs=seedT_sb[:, k, :],
                        start=(k == 0), stop=(k == K - 1))
            for m in range(K):
                nc.scalar.activation(
                    pre0_sb[:, m, :], psp[:, m, :],
                    mybir.ActivationFunctionType.Identity, bias=bias(0, m))

            # ---- main wavefront: tick t runs d0 step t, d1 step t-1, d2 step t-2
            # All d2 outputs accumulate in one big SBUF tile, leaving in a few
            # large SWDGE DMAs (few DMA lanes keeps the tail drain legal).
            u2all = consts.tile([P, S, K, T], BF16)
            u0, u1 = {}, {}
            u0[-1] = zeros
            u1[-1] = zeros

            def rec_mms(ps, d, u_in, u_prev):
                """ps[:,m,:] = (Wih[d] @ u_in + Whh[d] @ u_prev) tiles."""
                for m in range(K):
                    first = True
                    if u_in is not None:
                        for k in range(K):
                            nc.tensor.matmul(ps[:, m, :], lhsT=wih(d, k, m),
                                             rhs=u_in[:, k, :],
                                             start=first, stop=False)
                            first = False
                    for k in range(K):
                        nc.tensor.matmul(ps[:, m, :], lhsT=whh(d, k, m),
                                         rhs=u_prev[:, k, :],
                                         start=first, stop=(k == K - 1))
                        first = False

            for t in range(S + 2):
                if t < S:
                    s = t
                    ps = ps0p.tile([P, K, T], F32, tag="ps0")
                    rec_mms(ps, 0, None, u0[s - 1])
                    u = u0p.tile([P, K, T], BF16, tag="u0")
                    for m in range(K):
                        nc.scalar.activation(u[:, m, :], ps[:, m, :], TANH,
                                             bias=pre0_sb[:, m, s:s + 1])
                    u0[s] = u
                if 1 <= t <= S:
                    s = t - 1
                    ps = ps1p.tile([P, K, T], F32, tag="ps1")
                    rec_mms(ps, 1, u0[s], u1[s - 1])
                    u = u1p.tile([P, K, T], BF16, tag="u1")
                    for m in range(K):
                        nc.scalar.activation(u[:, m, :], ps[:, m, :], TANH,
                                             bias=bias(1, m))
                    u1[s] = u
                if 2 <= t:
                    s = t - 2
                    ps = ps2p.tile([P, K, T], F32, tag="ps2")
                    u2_prev = zeros if s == 0 else u2all[:, s - 1, :, :]
                    rec_mms(ps, 2, u1[s], u2_prev)
                    for m in range(K):
                        nc.scalar.activation(u2all[:, s, m, :], ps[:, m, :], TANH,
                                             bias=bias(2, m))
                    if (s + 1) % OCHUNK == 0:
                        s0 = s + 1 - OCHUNK
                        nc.gpsimd.dma_start(
                            out=out_c[:, s0:s0 + OCHUNK, :, :],
                            in_=u2all[:, s0:s0 + OCHUNK, :, :])
                for dd in (u0, u1):
                    dd.pop(t - 4, None)

    return nc


def _wblob(seed, wT_ih, wT_hh):
    """Pack per-core bf16 constants into the [P, WCW] weights blob."""
    b = np.empty((P, WCW), NPBF16)
    # wihT[d, k*128+p, m] -> cols (d*K+k)*H + m
    b[:, W0:W0 + D * K * H] = (
        wT_ih.reshape(D, K, P, H).transpose(2, 0, 1, 3).reshape(P, D * K * H))
    b[:, W1:W1 + D * K * H] = (
        wT_hh.reshape(D, K, P, H).transpose(2, 0, 1, 3).reshape(P, D * K * H))
    b[:, S0:S0 + H] = seed
    b[:, I0:I0 + P] = np.eye(P, dtype=np.float32)
    return b


def _cblob(bs):
    """Pack per-core fp32 bias columns: bsum[d, m*128+p] -> col d*K + m."""
    return np.ascontiguousarray(
        bs.reshape(D, K, P).transpose(2, 0, 1).reshape(P, D * K))


def kernel(src, trg, Wx_ih, Wx_hh, bx_ih, bx_hh, Wy_ih, Wy_hh, by_ih, by_hh):
    if "nc" not in _cache:
        _cache["nc"] = _build()
    nc = _cache["nc"]

    def tr(w):  # [D,H,H] -> W[d].T contiguous
        return np.ascontiguousarray(np.swapaxes(np.asarray(w, np.float32), 1, 2))

    src = np.asarray(src, np.float32)
    trg = np.asarray(trg, np.float32)
    wx_ihT, wx_hhT = tr(Wx_ih), tr(Wx_hh)
    wy_ihT, wy_hhT = tr(Wy_ih), tr(Wy_hh)
    bx = np.asarray(bx_ih, np.float32) + np.asarray(bx_hh, np.float32)
    by = np.asarray(by_ih, np.float32) + np.asarray(by_hh, np.float32)

    in_maps = []
    for b in range(B):  # cores 0-3: x chains
        in_maps.append({"wblob": _wblob(src[b], wx_ihT, wx_hhT),
                        "cblob": _cblob(bx)})
    for b in range(B):  # cores 4-7: y chains
        in_maps.append({"wblob": _wblob(trg[b], wy_ihT, wy_hhT),
                        "cblob": _cblob(by)})

    _cache["last_in_maps"] = in_maps
    globals()["_last_in_maps"] = in_maps
    res = run_bass_kernel_spmd(nc, in_maps, list(range(8)))

    out = np.empty((B, S, T, 2, H), np.float32)
    ii = np.arange(S)[:, None]
    jj = np.arange(T)[None, :]
    idx = (jj - ii) % T  # hx[i,j] = u_i[(j-i)%T]
    for b in range(B):
        # raw core output [p, s, k, v] -> [s, H=k*128+p, v]
        arr = np.asarray(res.results[b]["out"]).astype(np.float32)
        arr = arr.transpose(1, 2, 0, 3).reshape(S, H, T)
        hx = np.take_along_axis(arr, idx[:, None, :], axis=2)  # [s, H, j]
        out[b, :, :, 0, :] = hx.transpose(0, 2, 1)
        arr = np.asarray(res.results[B + b]["out"]).astype(np.float32)
        arr = arr.transpose(1, 2, 0, 3).reshape(S, H, T)
        out[b, :, :, 1, :] = arr.transpose(2, 0, 1)  # [j, H, i] -> [i, j, H]
    return out
